# revision 78
# baseline (speedup 1.0000x reference)
"""GCN-Cat message-passing kernel for 8 trn2 NeuronCores.

Strategy (v2):
  - GCNConv is linear before relu: aggregate features over edges, then apply W.
    With the concat structure, each layer only aggregates the newly produced
    features (8 / 64 / 128 dims).
  - Layer-1 aggregation (A @ inp) is a pure function of the inputs -> computed
    on HOST (bincount segment-sum). Device layer 1 is just 51 block FFNs.
  - Nodes relabeled so graphs are contiguous + padded to 128-multiples, whole
    graphs assigned to cores -> per-graph max-pool is a per-window max plus a
    masked AllReduce(max).
  - Edges sharded by dst core/block; segment-sum via one-hot matmuls on PE
    (PSUM accumulate). The one-hot is HOST-precomputed in fp8 and streamed
    from DRAM (no on-device is_equal work).
  - Per-edge gathers via gpsimd.dma_gather (256B rows). h1 table is bf16
    hi/lo pairs (~fp32); h2 table is fp16 (4e-3 end-to-end rel err).
  - Padded gather indices are -1: the Q7 ucode trims trailing negatives.
    Msg tiles are memset once at start so untouched slots stay finite
    (one-hot is 0 there, so they contribute nothing).
  - split == NCORES*HL0 so each gather half depends on exactly one of the
    two staged AllGathers.
"""
import contextlib
import sys

import ml_dtypes
import numpy as np

sys.path.insert(0, '/opt/trn_rl_repo')

import concourse.bacc as bacc
import concourse.mybir as mybir
import concourse.tile as tile
from concourse.library_config import mlp

BF16 = ml_dtypes.bfloat16
FP8 = ml_dtypes.float8_e4m3
NCORES = 8
P = 128
MAX_CHUNKS_PER_GATHER = 16  # 2048 idxs/op verified on HW (single_packet=False)

OH_DT = 'fp8'        # 'fp8' | 'bf16'  (one-hot storage dtype)
TRIM_PAD = True      # pad gather idx tails with -1 + per-core valid-count regs
AHEAD_BLOCKS = 5     # lead pieces of this many blocks emitted first per layer


def _ceil(a, b):
    return int(-(-a // b))


class Meta:
    pass


def preprocess(inputs, G=32):
    """Host-side prep: relabel nodes, shard/sort/pad edges, build per-core arrays."""
    norm, pos, x = (np.asarray(inputs[k]) for k in ('norm', 'pos', 'x'))
    edge_index = np.asarray(inputs['edge_index'])
    batch = np.asarray(inputs['batch']).astype(np.int64)
    N = norm.shape[0]

    inp = np.concatenate([norm, pos, x], axis=1).astype(np.float32)  # [N, 8]
    esrc = edge_index[0].astype(np.int64)
    edst = edge_index[1].astype(np.int64)

    # host segment-sum: agg0 = A @ inp  (first-layer aggregation)
    agg0 = np.stack([
        np.bincount(edst, weights=inp[esrc, f], minlength=N).astype(np.float32)
        for f in range(inp.shape[1])], axis=1)                       # [N, 8]

    counts = np.bincount(batch, minlength=G)
    starts = np.concatenate([[0], np.cumsum(counts)])
    gblocks = [_ceil(int(c), P) for c in counts]

    # assign graphs to cores, balancing padded block counts (LPT)
    core_blocks = [0] * NCORES
    core_graphs = [[] for _ in range(NCORES)]
    for g in sorted(range(G), key=lambda g: -gblocks[g]):
        k = int(np.argmin(core_blocks))
        core_blocks[k] += gblocks[g]
        core_graphs[k].append(g)
    B = max(max(core_blocks), 1)  # blocks per core (uniform)
    NLOC = B * P
    NFULL = NCORES * NLOC

    # node permutation + per-graph window map
    perm = np.zeros(N, np.int64)
    gwin = {}
    gowner = {}
    for k in range(NCORES):
        off = k * NLOC
        w = 0
        for g in core_graphs[k]:
            n = int(counts[g])
            if n == 0:
                continue
            perm[starts[g]:starts[g + 1]] = off + w * P + np.arange(n)
            gwin[g] = (k, w, w + _ceil(n, P))
            gowner[g] = k
            w += _ceil(n, P)

    src = perm[esrc]
    dst = perm[edst]

    # stage-major table index: tables laid out [stage0: 8 x HL0 | stage1: 8 x HL1]
    B0 = _ceil(B, 2)            # stage-0 blocks per core
    HL0, HL1 = B0 * P, (B - B0) * P
    split = NCORES * HL0        # gather-half boundary == AG stage boundary
    assert split <= 32768 and (NFULL - split) <= 32768
    kk = np.arange(NFULL) // NLOC
    rr = np.arange(NFULL) % NLOC
    tidx_map = np.where(rr < HL0,
                        kk * HL0 + rr,
                        NCORES * HL0 + kk * HL1 + (rr - HL0)).astype(np.int64)
    tsrc = tidx_map[src]

    blk = dst // P
    half = (tsrc >= split).astype(np.int64)
    order = np.lexsort((tsrc, half, blk))
    src_s, dst_s = src[order], dst[order]
    tsrc_s = tsrc[order]
    key_s = blk[order] * 2 + half[order]

    cnt = np.bincount(key_s, minlength=NCORES * B * 2).reshape(NCORES, B, 2)
    capL = np.array([_ceil(int(v), P) for v in cnt[:, :, 0].max(axis=0)])
    capH = np.array([_ceil(int(v), P) for v in cnt[:, :, 1].max(axis=0)])
    for b in range(B):
        if capL[b] + capH[b] == 0:
            capL[b] = 1

    # piece structure (gather granularity), identical on all cores
    pieces = []          # (block, half, n_chunks)
    piece_chunk_off = []  # (global chunk offset, chunks already consumed in (b,h))
    coff = 0
    for b in range(B):
        for h, cap in ((0, int(capL[b])), (1, int(capH[b]))):
            c0 = 0
            c = cap
            while c > 0:
                take = min(c, MAX_CHUNKS_PER_GATHER)
                pieces.append((b, h, take))
                piece_chunk_off.append((coff, c0))
                coff += take
                c0 += take
                c -= take
    tot_chunks = coff

    eoff = np.concatenate([[0], np.cumsum(np.bincount(
        key_s, minlength=NCORES * B * 2))]).astype(np.int64)

    agg0_full = np.zeros((NFULL, 8), np.float32)
    agg0_full[perm] = agg0

    oh_np = BF16 if OH_DT == 'bf16' else FP8
    eye = np.zeros((256, P), oh_np)
    eye[np.arange(P), np.arange(P)] = 1.0

    cores = []
    for k in range(NCORES):
        slot_tsrc = np.full(tot_chunks * P, -1 if TRIM_PAD else 0, np.int64)
        dst_vals = np.full(tot_chunks * P, 255, np.int64)
        piece_cnt = np.zeros(len(pieces), np.int32)
        for pi, (b, h, pc) in enumerate(pieces):
            cg, c0 = piece_chunk_off[pi]
            key = (k * B + b) * 2 + h
            s0, s1 = int(eoff[key]), int(eoff[key + 1])
            a = s0 + c0 * P
            bnd = min(s1, s0 + (c0 + pc) * P)
            n_here = max(0, bnd - a)
            piece_cnt[pi] = n_here
            if n_here > 0:
                sl = slice(cg * P, cg * P + n_here)
                slot_tsrc[sl] = tsrc_s[a:bnd]
                dst_vals[sl] = dst_s[a:bnd] % P
        idx_parts = []
        for pi, (b, h, pc) in enumerate(pieces):
            cg, _ = piece_chunk_off[pi]
            ids = slot_tsrc[cg * P:(cg + pc) * P].copy()
            if h == 1:
                ids = np.where(ids >= 0, ids - split, ids)
            if not TRIM_PAD:
                ids[ids < 0] = 0
            lay = ids.astype(np.int32).reshape(pc * 8, 16).T.astype(np.int16)
            idx_parts.append(np.tile(lay, (8, 1)))
        # one-hot [P, TC*P]: oh[p, c*P + q] = 1 iff slot c*128+p targets dst q
        dv = dst_vals.reshape(tot_chunks, P)          # [c, p]
        oh = eye[dv]                                  # [c, p, P]
        oh = np.ascontiguousarray(oh.transpose(1, 0, 2).reshape(P, tot_chunks * P))
        gmask = np.zeros((P, G), np.float32)
        for g, kk_ in gowner.items():
            if kk_ == k:
                gmask[:, g] = 1.0
        cores.append(dict(
            idx=np.ascontiguousarray(np.concatenate(idx_parts, axis=1)),
            oh=oh,
            agg0T=np.ascontiguousarray(
                agg0_full[k * NLOC:(k + 1) * NLOC].T.astype(np.float32)),
            gmask=gmask,
            gcnt=piece_cnt[None, :].copy(),
        ))

    W1, b1 = np.asarray(inputs['W1'], np.float32), np.asarray(inputs['b1'], np.float32)
    W2, b2 = np.asarray(inputs['W2'], np.float32), np.asarray(inputs['b2'], np.float32)
    W3, b3 = np.asarray(inputs['W3'], np.float32), np.asarray(inputs['b3'], np.float32)
    Wl, bl = np.asarray(inputs['Wl'], np.float32), np.asarray(inputs['bl'], np.float32)
    F1, F2, F3, C = W1.shape[1], W2.shape[1], W3.shape[1], Wl.shape[1]
    # stack row layout: stack1 = [A1(F1) | A0(8) | ones] ; stack2 = [A2(F2)]
    w1eff = np.concatenate([W1, b1[None, :]], 0)                       # [9, F1]
    w2eff = np.concatenate([W2[:F1], W2[F1:F1 + 8], b2[None, :]], 0)   # [F1+9, F2]
    w3a = np.concatenate([W3[:F1], W3[F1:F1 + 8] + W3[F1 + 8 + F2:],
                          b3[None, :]], 0)                             # [F1+9, F3]
    w3b = W3[F1 + 8:F1 + 8 + F2]                                       # [F2, F3]

    m = Meta()
    m.G, m.C, m.split = G, C, split
    m.B, m.NLOC, m.NFULL = B, NLOC, NFULL
    m.F1, m.F2, m.F3 = F1, F2, F3
    m.pieces, m.piece_chunk_off, m.tot_chunks = pieces, piece_chunk_off, tot_chunks
    m.capL, m.capH, m.cnt = capL, capH, cnt
    m.gwin, m.perm = gwin, perm
    m.B0, m.HL0, m.HL1 = B0, HL0, HL1
    m.maxpc = max(pc for _, _, pc in pieces)
    m.weights = dict(w1eff=w1eff, w2eff=w2eff, w3a=w3a, w3b=w3b, wl=Wl,
                     bl=bl[None, :].astype(np.float32))
    m.cores = cores
    return m


MSG2_BUFS = 12
MSG3_BUFS = 10
OH_BUFS = 10


def build(m):
    """Build the SPMD Tile program (identical across cores)."""
    fp32, bf16, fp16, i16 = (mybir.dt.float32, mybir.dt.bfloat16,
                             mybir.dt.float16, mybir.dt.int16)
    oh_dt = bf16 if OH_DT == 'bf16' else mybir.dt.float8e4
    F1, F2, F3, B, G, C = m.F1, m.F2, m.F3, m.B, m.G, m.C
    NLOC, NFULL, TC = m.NLOC, m.NFULL, m.tot_chunks
    KA = F1 + 9        # stack1 active rows (A1, A0, ones)
    FH = F3 // 2
    AF = mybir.ActivationFunctionType

    nc = bacc.Bacc("TRN2", target_bir_lowering=False, debug=False,
                   num_devices=NCORES, num_swdge_queues=4)

    NPIECES = len(m.pieces)
    p_idx = nc.dram_tensor("idxb", [P, TC * 8], i16, kind="ExternalInput")
    p_gcnt = nc.dram_tensor("gcnt", [1, NPIECES], mybir.dt.int32,
                            kind="ExternalInput")
    p_oh = nc.dram_tensor("ohb", [P, TC * P], oh_dt, kind="ExternalInput")
    p_agg0 = nc.dram_tensor("agg0T", [8, NLOC], fp32, kind="ExternalInput")
    p_gmask = nc.dram_tensor("gmask", [P, G], fp32, kind="ExternalInput")
    p_w1 = nc.dram_tensor("w1eff", [9, F1], fp32, kind="ExternalInput")
    p_w2 = nc.dram_tensor("w2eff", [KA, F2], fp32, kind="ExternalInput")
    p_w3a = nc.dram_tensor("w3a", [KA, F3], fp32, kind="ExternalInput")
    p_w3b = nc.dram_tensor("w3b", [F2, F3], fp32, kind="ExternalInput")
    p_wl = nc.dram_tensor("wl", [F3, C], fp32, kind="ExternalInput")
    p_bl = nc.dram_tensor("bl", [1, C], fp32, kind="ExternalInput")
    o_out = nc.dram_tensor("o_out", [G, C], fp32, kind="ExternalOutput")
    o_pred = nc.dram_tensor("o_pred", [G, C], fp32, kind="ExternalOutput")

    # stage-0/stage-1 tables are SEPARATE tensors: the tile framework tracks
    # DRAM deps per-tensor, so a shared tensor would serialize each gather
    # against both AllGathers (and epilogue stores against the stage-0 AG read)
    HL0, HL1 = m.HL0, m.HL1
    h1_loc = [nc.dram_tensor("h1_loc_a", [HL0, 2 * F1], bf16),
              nc.dram_tensor("h1_loc_b", [HL1, 2 * F1], bf16)]
    h2_loc = [nc.dram_tensor("h2_loc_a", [HL0, F2], fp16),
              nc.dram_tensor("h2_loc_b", [HL1, F2], fp16)]
    h1_full = [nc.dram_tensor("h1_full_a", [NCORES * HL0, 2 * F1], bf16,
                              addr_space="Shared"),
               nc.dram_tensor("h1_full_b", [NCORES * HL1, 2 * F1], bf16,
                              addr_space="Shared")]
    h2_full = [nc.dram_tensor("h2_full_a", [NCORES * HL0, F2], fp16,
                              addr_space="Shared"),
               nc.dram_tensor("h2_full_b", [NCORES * HL1, F2], fp16,
                              addr_space="Shared")]
    pool_loc = nc.dram_tensor("pool_loc", [2, P, G], fp32)
    pool_full = nc.dram_tensor("pool_full", [NCORES, 2, P, G], fp32,
                               addr_space="Shared")
    warm_loc = nc.dram_tensor("warm_loc", [1, 64], fp32)
    warm_full = nc.dram_tensor("warm_full", [NCORES, 64], fp32,
                               addr_space="Shared")

    rg = [list(range(NCORES))]

    with tile.TileContext(nc) as tc:
        nc.gpsimd.load_library(mlp)
        # warm up the collectives engine immediately: the first collective of
        # a NEFF carries ~40-80us one-time overhead; pay it during layer 1.
        nc.gpsimd.collective_compute(
            "AllGather", mybir.AluOpType.bypass, replica_groups=rg,
            ins=[warm_loc.ap().opt()], outs=[warm_full.ap().opt()])
        with contextlib.ExitStack() as ctx:
            const = ctx.enter_context(tc.tile_pool(name="const", bufs=1))
            ohp = ctx.enter_context(tc.tile_pool(name="oh", bufs=OH_BUFS))
            msgp = ctx.enter_context(tc.tile_pool(name="msg", bufs=7))
            hstp = ctx.enter_context(tc.tile_pool(name="hst", bufs=4))
            accp = ctx.enter_context(tc.tile_pool(name="acc", bufs=AHEAD_BLOCKS + 1,
                                                  space="PSUM"))
            finp = ctx.enter_context(tc.tile_pool(name="fin", bufs=2, space="PSUM"))

            idx_sb = const.tile([P, TC * 8], i16)
            gcnt_sb = const.tile([1, NPIECES], mybir.dt.int32)
            stack1 = const.tile([P, B * P], fp32)
            stack2 = const.tile([P, B * P], fp32)
            w1_sb = const.tile([P, F1], fp32)  # rows F1:F1+9 hold w1eff (base-64 match)
            w2_sb = const.tile([KA, F2], fp32)
            w3a_sb = [const.tile([KA, FH], fp32, tag=f"w3a{fh}", name=f"w3a{fh}") for fh in range(2)]
            w3b_sb = [const.tile([F2, FH], fp32, tag=f"w3b{fh}", name=f"w3b{fh}") for fh in range(2)]
            wl_sb = [const.tile([FH, C], fp32, tag=f"wl{fh}", name=f"wl{fh}") for fh in range(2)]
            bl_sb = const.tile([1, C], fp32)
            gmask_sb = const.tile([P, G], fp32)
            wmax = [const.tile([P, B], fp32, tag=f"wmax{fh}", name=f"wmax{fh}") for fh in range(2)]
            pooled = [const.tile([P, G], fp32, tag=f"pool{fh}", name=f"pool{fh}") for fh in range(2)]
            pool_sb = const.tile([P, 2 * G], fp32)
            soft = const.tile([G, 6 * C + 8], fp32)
            ones_g = const.tile([1, G], fp32)

            nc.sync.dma_start(idx_sb[:], p_idx[:])
            nc.sync.dma_start(gcnt_sb[:], p_gcnt[:])
            # ones row at F1+8; 32-partition-aligned memset, then the A0 DMA
            # overwrites rows F1:F1+8 (tile framework serializes the overlap)
            nc.vector.memset(stack1[F1:F1 + 32, :], 1.0)
            nc.sync.dma_start(stack1[F1:F1 + 8, :], p_agg0[:])
            nc.sync.dma_start(gmask_sb[:], p_gmask[:])
            nc.sync.dma_start(w1_sb[F1:F1 + 9, :], p_w1[:])
            nc.sync.dma_start(w2_sb[:], p_w2[:])
            for fh in range(2):
                fsl = slice(fh * FH, (fh + 1) * FH)
                nc.sync.dma_start(w3a_sb[fh][:], p_w3a[:, fsl])
                nc.sync.dma_start(w3b_sb[fh][:], p_w3b[:, fsl])
                nc.sync.dma_start(wl_sb[fh][:], p_wl[fsl, :])
            nc.sync.dma_start(bl_sb[:], p_bl[:])
            nc.vector.memset(ones_g[:], 1.0)

            def msg_tile(layer):
                if layer == 2:
                    return msgp.tile([P, m.maxpc, 2 * F1], bf16, tag="msg2",
                                     bufs=MSG2_BUFS, name="m2_t")
                return msgp.tile([P, m.maxpc, F2], fp16, tag="msg3",
                                 bufs=MSG3_BUFS, name="m3_t")

            # zero msg tiles once: -1-trimmed gathers leave slots untouched and
            # the matmul multiplies them by the one-hot zeros; virgin SBUF
            # could hold NaNs, so make every buffer finite up-front.
            if TRIM_PAD:
                for _ in range(MSG2_BUFS):
                    t = msg_tile(2)
                    nc.vector.memset(t[:], 0.0)
                for _ in range(MSG3_BUFS):
                    t = msg_tile(3)
                    nc.vector.memset(t[:], 0.0)

            def make_pair(hs, F):
                """hi/lo bf16 split of fp32 hs[:, :F] -> pair [P, 2F] bf16."""
                pair = hstp.tile([P, 2 * F1], bf16, tag="pair", name="pair_t")
                nc.vector.tensor_copy(out=pair[:, :F], in_=hs[:, :F])
                hif = hstp.tile([P, F2], fp32, tag="hif", name="hif_t")
                nc.scalar.copy(hif[:, :F], pair[:, :F])
                nc.vector.tensor_tensor(out=hif[:, :F], in0=hs[:, :F],
                                        in1=hif[:, :F],
                                        op=mybir.AluOpType.subtract)
                nc.vector.tensor_copy(out=pair[:, F:2 * F], in_=hif[:, :F])
                return pair

            def layer1_block(b):
                cols = slice(b * P, (b + 1) * P)
                # accp is idle during layer 1 -> use it for FFN pipelining
                h = accp.tile([P, P], fp32, tag="acc", name="acc_t")
                nc.tensor.matmul(h[:, :F1], stack1[F1:F1 + 9, cols],
                                 w1_sb[F1:F1 + 9, :], start=True, stop=True)
                hs = hstp.tile([P, F2], fp32, tag="hst", name="hst_t")
                nc.scalar.activation(hs[:, :F1], h[:, :F1], AF.Relu)
                pair = make_pair(hs, F1)
                # stores go on the sync queue (which carries nothing else hot);
                # the oh stream lives on the scalar queue so epilogue stores
                # can never head-of-line block it.
                s, r = (0, b) if b < m.B0 else (1, b - m.B0)
                nc.sync.dma_start(h1_loc[s][r * P:(r + 1) * P, :],
                                  pair[:, :2 * F1])

            def epilogue(layer, b, acc):
                cols = slice(b * P, (b + 1) * P)
                if layer == 2:
                    # A1 = hi-sums + lo-sums
                    nc.vector.tensor_copy(out=stack1[0:F1, cols],
                                          in_=acc[F1:2 * F1, :])
                    nc.vector.tensor_tensor(out=stack1[0:F1, cols],
                                            in0=acc[:F1, :],
                                            in1=stack1[0:F1, cols],
                                            op=mybir.AluOpType.add)
                    h = finp.tile([P, P], fp32, tag="fin", name="fin_t")
                    nc.tensor.matmul(h[:], stack1[0:KA, cols], w2_sb[:],
                                     start=True, stop=True)
                    hs = hstp.tile([P, F2], fp32, tag="hst", name="hst_t")
                    nc.scalar.activation(hs[:], h[:], AF.Relu)
                    pair16 = hstp.tile([P, F2], fp16, tag="p16", name="p16_t")
                    nc.vector.tensor_copy(out=pair16[:], in_=hs[:])
                    sr, r = (0, b) if b < m.B0 else (1, b - m.B0)
                    nc.sync.dma_start(h2_loc[sr][r * P:(r + 1) * P, :],
                                      pair16[:])
                else:
                    nc.vector.tensor_copy(out=stack2[:, cols], in_=acc[:, :])
                    for fh in range(2):
                        h3 = finp.tile([P, P], fp32, tag="fin", name="fin_t")
                        nc.tensor.matmul(h3[:], w3a_sb[fh][:], stack1[0:KA, cols],
                                         start=True, stop=False)
                        nc.tensor.matmul(h3[:], w3b_sb[fh][:], stack2[:, cols],
                                         start=False, stop=True)
                        hr = hstp.tile([P, P], fp32, tag="hst", name="hst3_t")
                        nc.scalar.activation(hr[:], h3[:], AF.Relu)
                        nc.vector.reduce_max(out=wmax[fh][:, b:b + 1], in_=hr[:],
                                             axis=mybir.AxisListType.X)
                    # per-graph window max as soon as the last window block of
                    # a graph is done -- keeps it off the final serial tail
                    for g in pool_trig.get(b, ()):
                        _, w0, w1 = m.gwin[g]
                        for fh in range(2):
                            nc.vector.reduce_max(
                                out=pooled[fh][:, g:g + 1],
                                in_=wmax[fh][:, w0:w1],
                                axis=mybir.AxisListType.X)

            def ag_part(loc, full, s):
                nc.gpsimd.collective_compute(
                    "AllGather", mybir.AluOpType.bypass, replica_groups=rg,
                    ins=[loc[s].ap().opt()], outs=[full[s].ap().opt()])

            # Per-layer piece orders. Lead with pieces of one gather-half from
            # a few blocks whose that-half work is largest on their LIGHTEST
            # core (trim empties small pieces), so the first collective wait
            # behind them is covered with real work on every core. Their
            # other-half pieces follow immediately so their PSUM accs release
            # before the rotation wraps (bufs = AHEAD_BLOCKS + 1).
            by_bh = {}
            for pi, (b, h, _) in enumerate(m.pieces):
                by_bh.setdefault((b, h), []).append(pi)

            def mk_order(lead_half, blkseq):
                hmin = m.cnt[:, :, lead_half].min(axis=0)
                A = [int(x) for x in np.argsort(-hmin)[:AHEAD_BLOCKS]]
                o = []
                for b in A:
                    o += by_bh.get((b, lead_half), [])
                for b in A:
                    o += by_bh.get((b, 1 - lead_half), [])
                for b in blkseq:
                    if b not in A:
                        o += by_bh.get((b, 0), []) + by_bh.get((b, 1), [])
                return o

            # L2: stage-1 blocks first so AG1(h2) runs mid-pass (hidden);
            # stage-0 completes last, its AG covered by L3's h1-lead pieces.
            order2 = mk_order(0, list(range(m.B0, B)) + list(range(m.B0)))
            order3 = mk_order(1, list(range(B)))

            # block completion positions in layer 3 -> per-graph pool trigger
            done_cnt, comp_pos = {}, {}
            for pi in order3:
                b, _, pc = m.pieces[pi]
                done_cnt[b] = done_cnt.get(b, 0) + pc
                if done_cnt[b] == int(m.capL[b] + m.capH[b]):
                    comp_pos[b] = len(comp_pos)
            pool_trig = {}
            for g in range(m.G):
                _, w0, w1 = m.gwin[g]
                tb = max(range(w0, w1), key=lambda x: comp_pos[x])
                pool_trig.setdefault(tb, []).append(g)

            # one reusable register for the per-piece valid-idx count: the
            # gpsimd SEQ is in-order, so reload-before-gather is race-free.
            cnt_reg = nc.gpsimd.alloc_register("gather_cnt") if TRIM_PAD else None

            def layer_pass(layer, order, ag=None):
                table = h1_full if layer == 2 else h2_full
                F = 2 * F1 if layer == 2 else F2
                state = {}  # block -> [acc tile, chunks done]
                s_done = [0, 0]

                for oi, pi in enumerate(order):
                    b, h, pc = m.pieces[pi]
                    cg, _ = m.piece_chunk_off[pi]
                    if b not in state:
                        state[b] = [accp.tile([P, P], fp32, tag="acc",
                                              name="acc_t"), 0]
                    acc, done = state[b]
                    cap_tot = int(m.capL[b] + m.capH[b])
                    msg = msg_tile(layer)
                    src_ap = table[h].ap()
                    if TRIM_PAD:
                        nc.gpsimd.reg_load(cnt_reg, gcnt_sb[0:1, pi:pi + 1])
                        cnt = cnt_reg
                    else:
                        cnt = pc * P
                    nc.gpsimd.dma_gather(
                        msg[:, :pc, :], src_ap,
                        idx_sb[:, cg * 8:(cg + pc) * 8],
                        pc * P, cnt, F,
                        queue_num=oi % 4, single_packet=False)
                    oh = ohp.tile([P, m.maxpc, P], oh_dt, tag="oh", name="oh_t")
                    nc.scalar.dma_start(
                        oh[:, :pc, :],
                        p_oh[:, cg * P:(cg + pc) * P].rearrange(
                            "p (c q) -> p c q", q=P))
                    for c in range(pc):
                        nc.tensor.matmul(
                            acc[0:F, :], msg[:, c, :], oh[:, c, :],
                            start=(done == 0),
                            stop=(done == cap_tot - 1))
                        done += 1
                    state[b][1] = done
                    if done == cap_tot:
                        epilogue(layer, b, acc)
                        del state[b]
                        if ag is not None:
                            sr = 0 if b < m.B0 else 1
                            s_done[sr] += 1
                            if s_done[sr] == (m.B0 if sr == 0 else B - m.B0):
                                ag_part(ag[0], ag[1], sr)
                assert not state

            for b in range(B):
                layer1_block(b)
                if b == m.B0 - 1:
                    ag_part(h1_loc, h1_full, 0)
                elif b == B - 1:
                    ag_part(h1_loc, h1_full, 1)
            layer_pass(2, order2, ag=(h2_loc, h2_full))
            layer_pass(3, order3)

            # per-graph window maxes were emitted inside layer 3; finish with
            # mask -> store -> AllGather -> cross-core max.
            for fh in range(2):
                nc.vector.tensor_tensor(out=pooled[fh][:], in0=pooled[fh][:],
                                        in1=gmask_sb[:],
                                        op=mybir.AluOpType.mult)
                nc.sync.dma_start(pool_loc[fh, :, :], pooled[fh][:])
            nc.gpsimd.collective_compute(
                "AllGather", mybir.AluOpType.bypass, replica_groups=rg,
                ins=[pool_loc.ap().opt()], outs=[pool_full.ap().opt()])
            pw = const.tile([P, 2 * NCORES, G], fp32)
            nc.sync.dma_start(
                pw[:],
                pool_full.ap().rearrange("k i p g -> p (k i) g"))
            for fh in range(2):
                dstc = pool_sb[:, fh * G:(fh + 1) * G]
                nc.vector.tensor_copy(out=dstc, in_=pw[:, fh, :])
                for k in range(1, NCORES):
                    nc.vector.tensor_tensor(out=dstc, in0=dstc,
                                            in1=pw[:, 2 * k + fh, :],
                                            op=mybir.AluOpType.max)

            lg = finp.tile([P, C], fp32, tag="fin", name="lg_t")
            nc.tensor.matmul(lg[:G, :], pool_sb[:, 0:G], wl_sb[0][:],
                             start=True, stop=False)
            nc.tensor.matmul(lg[:G, :], pool_sb[:, G:2 * G], wl_sb[1][:],
                             start=False, stop=False)
            nc.tensor.matmul(lg[:G, :], ones_g[:], bl_sb[:],
                             start=False, stop=True)

            z, zs = soft[:, 0:C], soft[:, C:2 * C]
            e, ot = soft[:, 2 * C:3 * C], soft[:, 3 * C:4 * C]
            pr = soft[:, 4 * C:5 * C]
            mx, sm = soft[:, 5 * C:5 * C + 1], soft[:, 5 * C + 1:5 * C + 2]
            ls, ri = soft[:, 5 * C + 2:5 * C + 3], soft[:, 5 * C + 3:5 * C + 4]
            nc.vector.tensor_copy(out=z, in_=lg[:G, :])
            nc.vector.reduce_max(out=mx, in_=z, axis=mybir.AxisListType.X)
            nc.vector.tensor_scalar(out=zs, in0=z, scalar1=mx, scalar2=None,
                                    op0=mybir.AluOpType.subtract)
            nc.scalar.activation(e, zs, AF.Exp)
            nc.vector.reduce_sum(out=sm, in_=e, axis=mybir.AxisListType.X)
            nc.scalar.activation(ls, sm, AF.Ln)
            nc.vector.reciprocal(ri, sm)
            nc.vector.tensor_scalar(out=ot, in0=zs, scalar1=ls, scalar2=None,
                                    op0=mybir.AluOpType.subtract)
            nc.vector.tensor_scalar(out=pr, in0=e, scalar1=ri, scalar2=None,
                                    op0=mybir.AluOpType.mult)
            nc.sync.dma_start(o_out[:], ot)
            nc.sync.dma_start(o_pred[:], pr)

    nc.compile()
    return nc


def make_in_maps(m):
    w = m.weights
    shared = {"w1eff": w['w1eff'], "w2eff": w['w2eff'], "w3a": w['w3a'],
              "w3b": w['w3b'], "wl": w['wl'], "bl": w['bl']}
    return [{**shared, "idxb": c['idx'], "ohb": c['oh'], "gcnt": c['gcnt'],
             "agg0T": c['agg0T'], "gmask": c['gmask']} for c in m.cores]


def run(inputs, G=32, trace=False):
    from concourse.bass_utils import run_bass_kernel_spmd
    m = preprocess(inputs, G=G)
    nc = build(m)
    maps = make_in_maps(m)
    res = run_bass_kernel_spmd(nc, maps, list(range(NCORES)), trace=trace)
    out = np.asarray(res.results[0]["o_out"])
    pred = np.asarray(res.results[0]["o_pred"])
    return (out, pred), res


def kernel(**inputs):
    """Full-inputs -> full-output GCN forward on 8 trn2 NeuronCores."""
    from concourse.bass_utils import run_bass_kernel_spmd
    m = preprocess(inputs, G=32)
    nc = build(m)
    maps = make_in_maps(m)
    res = run_bass_kernel_spmd(nc, maps, list(range(NCORES)), trace=False)
    out = np.asarray(res.results[0]["o_out"], dtype=np.float32)
    pred = np.asarray(res.results[0]["o_pred"], dtype=np.float32)
    return (out, pred)


# revision 84
# speedup vs baseline: 1.0117x; 1.0117x over previous
"""GCN-Cat message-passing kernel for 8 trn2 NeuronCores.

Strategy (v2):
  - GCNConv is linear before relu: aggregate features over edges, then apply W.
    With the concat structure, each layer only aggregates the newly produced
    features (8 / 64 / 128 dims).
  - Layer-1 aggregation (A @ inp) is a pure function of the inputs -> computed
    on HOST (bincount segment-sum). Device layer 1 is just 51 block FFNs.
  - Nodes relabeled so graphs are contiguous + padded to 128-multiples, whole
    graphs assigned to cores -> per-graph max-pool is a per-window max plus a
    masked AllReduce(max).
  - Edges sharded by dst core/block; segment-sum via one-hot matmuls on PE
    (PSUM accumulate). The one-hot is HOST-precomputed in fp8 and streamed
    from DRAM (no on-device is_equal work).
  - Per-edge gathers via gpsimd.dma_gather (256B rows). h1 table is bf16
    hi/lo pairs (~fp32); h2 table is fp16 (4e-3 end-to-end rel err).
  - Padded gather indices are -1: the Q7 ucode trims trailing negatives.
    Msg tiles are memset once at start so untouched slots stay finite
    (one-hot is 0 there, so they contribute nothing).
  - split == NCORES*HL0 so each gather half depends on exactly one of the
    two staged AllGathers.
"""
import contextlib
import sys

import ml_dtypes
import numpy as np

sys.path.insert(0, '/opt/trn_rl_repo')

import concourse.bacc as bacc
import concourse.mybir as mybir
import concourse.tile as tile
from concourse.library_config import mlp

BF16 = ml_dtypes.bfloat16
FP8 = ml_dtypes.float8_e4m3
NCORES = 8
P = 128
MAX_CHUNKS_PER_GATHER = 16  # 2048 idxs/op verified on HW (single_packet=False)

OH_DT = 'fp8'        # 'fp8' | 'bf16'  (one-hot storage dtype)
TRIM_PAD = True      # pad gather idx tails with -1 + per-core valid-count regs
AHEAD_BLOCKS = 5     # lead pieces of this many blocks emitted first per layer


def _ceil(a, b):
    return int(-(-a // b))


class Meta:
    pass


def preprocess(inputs, G=32):
    """Host-side prep: relabel nodes, shard/sort/pad edges, build per-core arrays."""
    norm, pos, x = (np.asarray(inputs[k]) for k in ('norm', 'pos', 'x'))
    edge_index = np.asarray(inputs['edge_index'])
    batch = np.asarray(inputs['batch']).astype(np.int64)
    N = norm.shape[0]

    inp = np.concatenate([norm, pos, x], axis=1).astype(np.float32)  # [N, 8]
    esrc = edge_index[0].astype(np.int64)
    edst = edge_index[1].astype(np.int64)

    # host segment-sum: agg0 = A @ inp  (first-layer aggregation)
    agg0 = np.stack([
        np.bincount(edst, weights=inp[esrc, f], minlength=N).astype(np.float32)
        for f in range(inp.shape[1])], axis=1)                       # [N, 8]

    counts = np.bincount(batch, minlength=G)
    starts = np.concatenate([[0], np.cumsum(counts)])
    gblocks = [_ceil(int(c), P) for c in counts]

    # assign graphs to cores, balancing padded block counts (LPT)
    core_blocks = [0] * NCORES
    core_graphs = [[] for _ in range(NCORES)]
    for g in sorted(range(G), key=lambda g: -gblocks[g]):
        k = int(np.argmin(core_blocks))
        core_blocks[k] += gblocks[g]
        core_graphs[k].append(g)
    B = max(max(core_blocks), 1)  # blocks per core (uniform)
    NLOC = B * P
    NFULL = NCORES * NLOC

    # node permutation + per-graph window map
    perm = np.zeros(N, np.int64)
    gwin = {}
    gowner = {}
    for k in range(NCORES):
        off = k * NLOC
        w = 0
        for g in core_graphs[k]:
            n = int(counts[g])
            if n == 0:
                continue
            perm[starts[g]:starts[g + 1]] = off + w * P + np.arange(n)
            gwin[g] = (k, w, w + _ceil(n, P))
            gowner[g] = k
            w += _ceil(n, P)

    src = perm[esrc]
    dst = perm[edst]

    # stage-major table index: tables laid out [stage0: 8 x HL0 | stage1: 8 x HL1]
    B0 = _ceil(B, 2)            # stage-0 blocks per core
    HL0, HL1 = B0 * P, (B - B0) * P
    split = NCORES * HL0        # gather-half boundary == AG stage boundary
    assert split <= 32768 and (NFULL - split) <= 32768
    kk = np.arange(NFULL) // NLOC
    rr = np.arange(NFULL) % NLOC
    tidx_map = np.where(rr < HL0,
                        kk * HL0 + rr,
                        NCORES * HL0 + kk * HL1 + (rr - HL0)).astype(np.int64)
    tsrc = tidx_map[src]

    blk = dst // P
    half = (tsrc >= split).astype(np.int64)
    order = np.lexsort((tsrc, half, blk))
    src_s, dst_s = src[order], dst[order]
    tsrc_s = tsrc[order]
    key_s = blk[order] * 2 + half[order]

    cnt = np.bincount(key_s, minlength=NCORES * B * 2).reshape(NCORES, B, 2)
    capL = np.array([_ceil(int(v), P) for v in cnt[:, :, 0].max(axis=0)])
    capH = np.array([_ceil(int(v), P) for v in cnt[:, :, 1].max(axis=0)])
    for b in range(B):
        if capL[b] + capH[b] == 0:
            capL[b] = 1

    # piece structure (gather granularity), identical on all cores
    pieces = []          # (block, half, n_chunks)
    piece_chunk_off = []  # (global chunk offset, chunks already consumed in (b,h))
    coff = 0
    for b in range(B):
        for h, cap in ((0, int(capL[b])), (1, int(capH[b]))):
            c0 = 0
            c = cap
            while c > 0:
                take = min(c, MAX_CHUNKS_PER_GATHER)
                pieces.append((b, h, take))
                piece_chunk_off.append((coff, c0))
                coff += take
                c0 += take
                c -= take
    tot_chunks = coff

    eoff = np.concatenate([[0], np.cumsum(np.bincount(
        key_s, minlength=NCORES * B * 2))]).astype(np.int64)

    agg0_full = np.zeros((NFULL, 8), np.float32)
    agg0_full[perm] = agg0

    oh_np = BF16 if OH_DT == 'bf16' else FP8
    eye = np.zeros((256, P), oh_np)
    eye[np.arange(P), np.arange(P)] = 1.0

    cores = []
    for k in range(NCORES):
        slot_tsrc = np.full(tot_chunks * P, -1 if TRIM_PAD else 0, np.int64)
        dst_vals = np.full(tot_chunks * P, 255, np.int64)
        piece_cnt = np.zeros(len(pieces), np.int32)
        for pi, (b, h, pc) in enumerate(pieces):
            cg, c0 = piece_chunk_off[pi]
            key = (k * B + b) * 2 + h
            s0, s1 = int(eoff[key]), int(eoff[key + 1])
            a = s0 + c0 * P
            bnd = min(s1, s0 + (c0 + pc) * P)
            n_here = max(0, bnd - a)
            piece_cnt[pi] = n_here
            if n_here > 0:
                sl = slice(cg * P, cg * P + n_here)
                slot_tsrc[sl] = tsrc_s[a:bnd]
                dst_vals[sl] = dst_s[a:bnd] % P
        idx_parts = []
        for pi, (b, h, pc) in enumerate(pieces):
            cg, _ = piece_chunk_off[pi]
            ids = slot_tsrc[cg * P:(cg + pc) * P].copy()
            if h == 1:
                ids = np.where(ids >= 0, ids - split, ids)
            if not TRIM_PAD:
                ids[ids < 0] = 0
            lay = ids.astype(np.int32).reshape(pc * 8, 16).T.astype(np.int16)
            idx_parts.append(np.tile(lay, (8, 1)))
        # one-hot [P, TC*P]: oh[p, c*P + q] = 1 iff slot c*128+p targets dst q
        dv = dst_vals.reshape(tot_chunks, P)          # [c, p]
        oh = eye[dv]                                  # [c, p, P]
        oh = np.ascontiguousarray(oh.transpose(1, 0, 2).reshape(P, tot_chunks * P))
        gmask = np.zeros((P, G), np.float32)
        for g, kk_ in gowner.items():
            if kk_ == k:
                gmask[:, g] = 1.0
        cores.append(dict(
            idx=np.ascontiguousarray(np.concatenate(idx_parts, axis=1)),
            oh=oh,
            agg0T=np.ascontiguousarray(
                agg0_full[k * NLOC:(k + 1) * NLOC].T.astype(np.float32)),
            gmask=gmask,
            gcnt=piece_cnt[None, :].copy(),
        ))

    W1, b1 = np.asarray(inputs['W1'], np.float32), np.asarray(inputs['b1'], np.float32)
    W2, b2 = np.asarray(inputs['W2'], np.float32), np.asarray(inputs['b2'], np.float32)
    W3, b3 = np.asarray(inputs['W3'], np.float32), np.asarray(inputs['b3'], np.float32)
    Wl, bl = np.asarray(inputs['Wl'], np.float32), np.asarray(inputs['bl'], np.float32)
    F1, F2, F3, C = W1.shape[1], W2.shape[1], W3.shape[1], Wl.shape[1]
    # stack row layout: stack1 = [A1(F1) | A0(8) | ones] ; stack2 = [A2(F2)]
    w1eff = np.concatenate([W1, b1[None, :]], 0)                       # [9, F1]
    w2eff = np.concatenate([W2[:F1], W2[F1:F1 + 8], b2[None, :]], 0)   # [F1+9, F2]
    w3a = np.concatenate([W3[:F1], W3[F1:F1 + 8] + W3[F1 + 8 + F2:],
                          b3[None, :]], 0)                             # [F1+9, F3]
    w3b = W3[F1 + 8:F1 + 8 + F2]                                       # [F2, F3]

    m = Meta()
    m.G, m.C, m.split = G, C, split
    m.B, m.NLOC, m.NFULL = B, NLOC, NFULL
    m.F1, m.F2, m.F3 = F1, F2, F3
    m.pieces, m.piece_chunk_off, m.tot_chunks = pieces, piece_chunk_off, tot_chunks
    m.capL, m.capH, m.cnt = capL, capH, cnt
    m.gwin, m.perm = gwin, perm
    m.B0, m.HL0, m.HL1 = B0, HL0, HL1
    m.maxpc = max(pc for _, _, pc in pieces)
    m.weights = dict(w1eff=w1eff, w2eff=w2eff, w3a=w3a, w3b=w3b, wl=Wl,
                     bl=bl[None, :].astype(np.float32))
    m.cores = cores
    return m


MSG2_BUFS = 12
MSG3_BUFS = 10
OH_BUFS = 10


def build(m):
    """Build the SPMD Tile program (identical across cores)."""
    fp32, bf16, fp16, i16 = (mybir.dt.float32, mybir.dt.bfloat16,
                             mybir.dt.float16, mybir.dt.int16)
    oh_dt = bf16 if OH_DT == 'bf16' else mybir.dt.float8e4
    F1, F2, F3, B, G, C = m.F1, m.F2, m.F3, m.B, m.G, m.C
    NLOC, NFULL, TC = m.NLOC, m.NFULL, m.tot_chunks
    KA = F1 + 9        # stack1 active rows (A1, A0, ones)
    FH = F3 // 2
    AF = mybir.ActivationFunctionType

    nc = bacc.Bacc("TRN2", target_bir_lowering=False, debug=False,
                   num_devices=NCORES, num_swdge_queues=4)

    NPIECES = len(m.pieces)
    p_idx = nc.dram_tensor("idxb", [P, TC * 8], i16, kind="ExternalInput")
    p_gcnt = nc.dram_tensor("gcnt", [1, NPIECES], mybir.dt.int32,
                            kind="ExternalInput")
    p_oh = nc.dram_tensor("ohb", [P, TC * P], oh_dt, kind="ExternalInput")
    p_agg0 = nc.dram_tensor("agg0T", [8, NLOC], fp32, kind="ExternalInput")
    p_gmask = nc.dram_tensor("gmask", [P, G], fp32, kind="ExternalInput")
    p_w1 = nc.dram_tensor("w1eff", [9, F1], fp32, kind="ExternalInput")
    p_w2 = nc.dram_tensor("w2eff", [KA, F2], fp32, kind="ExternalInput")
    p_w3a = nc.dram_tensor("w3a", [KA, F3], fp32, kind="ExternalInput")
    p_w3b = nc.dram_tensor("w3b", [F2, F3], fp32, kind="ExternalInput")
    p_wl = nc.dram_tensor("wl", [F3, C], fp32, kind="ExternalInput")
    p_bl = nc.dram_tensor("bl", [1, C], fp32, kind="ExternalInput")
    o_out = nc.dram_tensor("o_out", [G, C], fp32, kind="ExternalOutput")
    o_pred = nc.dram_tensor("o_pred", [G, C], fp32, kind="ExternalOutput")

    # stage-0/stage-1 tables are SEPARATE tensors: the tile framework tracks
    # DRAM deps per-tensor, so a shared tensor would serialize each gather
    # against both AllGathers (and epilogue stores against the stage-0 AG read)
    HL0, HL1 = m.HL0, m.HL1
    h1_loc = [nc.dram_tensor("h1_loc_a", [HL0, 2 * F1], bf16),
              nc.dram_tensor("h1_loc_b", [HL1, 2 * F1], bf16)]
    h2_loc = [nc.dram_tensor("h2_loc_a", [HL0, F2], fp16),
              nc.dram_tensor("h2_loc_b", [HL1, F2], fp16)]
    h1_full = [nc.dram_tensor("h1_full_a", [NCORES * HL0, 2 * F1], bf16,
                              addr_space="Shared"),
               nc.dram_tensor("h1_full_b", [NCORES * HL1, 2 * F1], bf16,
                              addr_space="Shared")]
    h2_full = [nc.dram_tensor("h2_full_a", [NCORES * HL0, F2], fp16,
                              addr_space="Shared"),
               nc.dram_tensor("h2_full_b", [NCORES * HL1, F2], fp16,
                              addr_space="Shared")]
    pool_loc = nc.dram_tensor("pool_loc", [2, P, G], fp32)
    pool_full = nc.dram_tensor("pool_full", [NCORES, 2, P, G], fp32,
                               addr_space="Shared")
    warm_loc = nc.dram_tensor("warm_loc", [1, 64], fp32)
    warm_full = nc.dram_tensor("warm_full", [NCORES, 64], fp32,
                               addr_space="Shared")

    rg = [list(range(NCORES))]

    with tile.TileContext(nc) as tc:
        nc.gpsimd.load_library(mlp)
        # warm up the collectives engine immediately: the first collective of
        # a NEFF carries ~40-80us one-time overhead; pay it during layer 1.
        nc.gpsimd.collective_compute(
            "AllGather", mybir.AluOpType.bypass, replica_groups=rg,
            ins=[warm_loc.ap().opt()], outs=[warm_full.ap().opt()])
        with contextlib.ExitStack() as ctx:
            const = ctx.enter_context(tc.tile_pool(name="const", bufs=1))
            ohp = ctx.enter_context(tc.tile_pool(name="oh", bufs=OH_BUFS))
            msgp = ctx.enter_context(tc.tile_pool(name="msg", bufs=7))
            hstp = ctx.enter_context(tc.tile_pool(name="hst", bufs=4))
            accp = ctx.enter_context(tc.tile_pool(name="acc", bufs=AHEAD_BLOCKS + 1,
                                                  space="PSUM"))
            finp = ctx.enter_context(tc.tile_pool(name="fin", bufs=2, space="PSUM"))

            idx_sb = const.tile([P, TC * 8], i16)
            gcnt_sb = const.tile([1, NPIECES], mybir.dt.int32)
            stack1 = const.tile([P, B * P], fp32)
            stack2 = const.tile([P, B * P], fp32)
            w1_sb = const.tile([P, F1], fp32)  # rows F1:F1+9 hold w1eff (base-64 match)
            w2_sb = const.tile([KA, F2], fp32)
            w3a_sb = [const.tile([KA, FH], fp32, tag=f"w3a{fh}", name=f"w3a{fh}") for fh in range(2)]
            w3b_sb = [const.tile([F2, FH], fp32, tag=f"w3b{fh}", name=f"w3b{fh}") for fh in range(2)]
            wl_sb = [const.tile([FH, C], fp32, tag=f"wl{fh}", name=f"wl{fh}") for fh in range(2)]
            bl_sb = const.tile([1, C], fp32)
            gmask_sb = const.tile([P, G], fp32)
            wmax = [const.tile([P, B], fp32, tag=f"wmax{fh}", name=f"wmax{fh}") for fh in range(2)]
            pooled = [const.tile([P, G], fp32, tag=f"pool{fh}", name=f"pool{fh}") for fh in range(2)]
            pool_sb = const.tile([P, 2 * G], fp32)
            soft = const.tile([G, 6 * C + 8], fp32)
            ones_g = const.tile([1, G], fp32)

            nc.sync.dma_start(idx_sb[:], p_idx[:])
            nc.sync.dma_start(gcnt_sb[:], p_gcnt[:])
            # ones row at F1+8; 32-partition-aligned memset, then the A0 DMA
            # overwrites rows F1:F1+8 (tile framework serializes the overlap)
            nc.vector.memset(stack1[F1:F1 + 32, :], 1.0)
            nc.sync.dma_start(stack1[F1:F1 + 8, :], p_agg0[:])
            nc.sync.dma_start(gmask_sb[:], p_gmask[:])
            nc.sync.dma_start(w1_sb[F1:F1 + 9, :], p_w1[:])
            nc.sync.dma_start(w2_sb[:], p_w2[:])
            for fh in range(2):
                fsl = slice(fh * FH, (fh + 1) * FH)
                nc.sync.dma_start(w3a_sb[fh][:], p_w3a[:, fsl])
                nc.sync.dma_start(w3b_sb[fh][:], p_w3b[:, fsl])
                nc.sync.dma_start(wl_sb[fh][:], p_wl[fsl, :])
            nc.sync.dma_start(bl_sb[:], p_bl[:])
            nc.vector.memset(ones_g[:], 1.0)

            def msg_tile(layer):
                if layer == 2:
                    return msgp.tile([P, m.maxpc, 2 * F1], bf16, tag="msg2",
                                     bufs=MSG2_BUFS, name="m2_t")
                return msgp.tile([P, m.maxpc, F2], fp16, tag="msg3",
                                 bufs=MSG3_BUFS, name="m3_t")

            # zero msg tiles once: -1-trimmed gathers leave slots untouched and
            # the matmul multiplies them by the one-hot zeros; virgin SBUF
            # could hold NaNs, so make every buffer finite up-front.
            if TRIM_PAD:
                for _ in range(MSG2_BUFS):
                    t = msg_tile(2)
                    nc.vector.memset(t[:], 0.0)
                for _ in range(MSG3_BUFS):
                    t = msg_tile(3)
                    nc.vector.memset(t[:], 0.0)

            def make_pair(hs, F):
                """hi/lo bf16 split of fp32 hs[:, :F] -> pair [P, 2F] bf16."""
                pair = hstp.tile([P, 2 * F1], bf16, tag="pair", name="pair_t")
                nc.vector.tensor_copy(out=pair[:, :F], in_=hs[:, :F])
                hif = hstp.tile([P, F2], fp32, tag="hif", name="hif_t")
                nc.scalar.copy(hif[:, :F], pair[:, :F])
                nc.vector.tensor_tensor(out=hif[:, :F], in0=hs[:, :F],
                                        in1=hif[:, :F],
                                        op=mybir.AluOpType.subtract)
                nc.vector.tensor_copy(out=pair[:, F:2 * F], in_=hif[:, :F])
                return pair

            def layer1_block(b):
                cols = slice(b * P, (b + 1) * P)
                # accp is idle during layer 1 -> use it for FFN pipelining
                h = accp.tile([P, P], fp32, tag="acc", name="acc_t")
                nc.tensor.matmul(h[:, :F1], stack1[F1:F1 + 9, cols],
                                 w1_sb[F1:F1 + 9, :], start=True, stop=True)
                hs = hstp.tile([P, F2], fp32, tag="hst", name="hst_t")
                nc.scalar.activation(hs[:, :F1], h[:, :F1], AF.Relu)
                pair = make_pair(hs, F1)
                # stores go on the sync queue (which carries nothing else hot);
                # the oh stream lives on the scalar queue so epilogue stores
                # can never head-of-line block it.
                s, r = (0, b) if b < m.B0 else (1, b - m.B0)
                nc.sync.dma_start(h1_loc[s][r * P:(r + 1) * P, :],
                                  pair[:, :2 * F1])

            def epilogue(layer, b, acc):
                cols = slice(b * P, (b + 1) * P)
                if layer == 2:
                    # A1 = hi-sums + lo-sums
                    nc.vector.tensor_copy(out=stack1[0:F1, cols],
                                          in_=acc[F1:2 * F1, :])
                    nc.vector.tensor_tensor(out=stack1[0:F1, cols],
                                            in0=acc[:F1, :],
                                            in1=stack1[0:F1, cols],
                                            op=mybir.AluOpType.add)
                    h = finp.tile([P, P], fp32, tag="fin", name="fin_t")
                    nc.tensor.matmul(h[:], stack1[0:KA, cols], w2_sb[:],
                                     start=True, stop=True)
                    hs = hstp.tile([P, F2], fp32, tag="hst", name="hst_t")
                    nc.scalar.activation(hs[:], h[:], AF.Relu)
                    pair16 = hstp.tile([P, F2], fp16, tag="p16", name="p16_t")
                    nc.vector.tensor_copy(out=pair16[:], in_=hs[:])
                    sr, r = (0, b) if b < m.B0 else (1, b - m.B0)
                    nc.sync.dma_start(h2_loc[sr][r * P:(r + 1) * P, :],
                                      pair16[:])
                else:
                    nc.vector.tensor_copy(out=stack2[:, cols], in_=acc[:, :])
                    for fh in range(2):
                        h3 = finp.tile([P, P], fp32, tag="fin", name="fin_t")
                        nc.tensor.matmul(h3[:], w3a_sb[fh][:], stack1[0:KA, cols],
                                         start=True, stop=False)
                        nc.tensor.matmul(h3[:], w3b_sb[fh][:], stack2[:, cols],
                                         start=False, stop=True)
                        hr = hstp.tile([P, P], fp32, tag="hst", name="hst3_t")
                        nc.scalar.activation(hr[:], h3[:], AF.Relu)
                        nc.vector.reduce_max(out=wmax[fh][:, b:b + 1], in_=hr[:],
                                             axis=mybir.AxisListType.X)
                    # per-graph window max as soon as the last window block of
                    # a graph is done -- keeps it off the final serial tail
                    for g in pool_trig.get(b, ()):
                        _, w0, w1 = m.gwin[g]
                        for fh in range(2):
                            nc.vector.reduce_max(
                                out=pooled[fh][:, g:g + 1],
                                in_=wmax[fh][:, w0:w1],
                                axis=mybir.AxisListType.X)

            def ag_part(loc, full, s):
                nc.gpsimd.collective_compute(
                    "AllGather", mybir.AluOpType.bypass, replica_groups=rg,
                    ins=[loc[s].ap().opt()], outs=[full[s].ap().opt()])

            # Per-layer piece orders. Lead with pieces of one gather-half from
            # a few blocks whose that-half work is largest on their LIGHTEST
            # core (trim empties small pieces), so the first collective wait
            # behind them is covered with real work on every core. Their
            # other-half pieces follow immediately so their PSUM accs release
            # before the rotation wraps (bufs = AHEAD_BLOCKS + 1).
            by_bh = {}
            for pi, (b, h, _) in enumerate(m.pieces):
                by_bh.setdefault((b, h), []).append(pi)

            def mk_order(lead_half, blkseq):
                hmin = m.cnt[:, :, lead_half].min(axis=0)
                A = [int(x) for x in np.argsort(-hmin)[:AHEAD_BLOCKS]]
                o = []
                for b in A:
                    o += by_bh.get((b, lead_half), [])
                for b in A:
                    o += by_bh.get((b, 1 - lead_half), [])
                for b in blkseq:
                    if b not in A:
                        o += by_bh.get((b, 0), []) + by_bh.get((b, 1), [])
                return o

            # L2: stage-1 blocks first so AG1(h2) runs mid-pass (hidden);
            # stage-0 completes last, its AG covered by L3's h1-lead pieces.
            order2 = mk_order(0, list(range(m.B0, B)) + list(range(m.B0)))
            order3 = mk_order(1, list(range(B)))

            # block completion positions in layer 3 -> per-graph pool trigger
            done_cnt, comp_pos = {}, {}
            for pi in order3:
                b, _, pc = m.pieces[pi]
                done_cnt[b] = done_cnt.get(b, 0) + pc
                if done_cnt[b] == int(m.capL[b] + m.capH[b]):
                    comp_pos[b] = len(comp_pos)
            pool_trig = {}
            for g in range(m.G):
                _, w0, w1 = m.gwin[g]
                tb = max(range(w0, w1), key=lambda x: comp_pos[x])
                pool_trig.setdefault(tb, []).append(g)

            # one reusable register for the per-piece valid-idx count: the
            # gpsimd SEQ is in-order, so reload-before-gather is race-free.
            cnt_reg = nc.gpsimd.alloc_register("gather_cnt") if TRIM_PAD else None

            def layer_pass(layer, order, ag=None):
                table = h1_full if layer == 2 else h2_full
                F = 2 * F1 if layer == 2 else F2
                state = {}  # block -> [acc tile, chunks done]
                s_done = [0, 0]

                for oi, pi in enumerate(order):
                    b, h, pc = m.pieces[pi]
                    cg, _ = m.piece_chunk_off[pi]
                    if b not in state:
                        state[b] = [accp.tile([P, P], fp32, tag="acc",
                                              name="acc_t"), 0]
                    acc, done = state[b]
                    cap_tot = int(m.capL[b] + m.capH[b])
                    msg = msg_tile(layer)
                    src_ap = table[h].ap()
                    if TRIM_PAD:
                        nc.gpsimd.reg_load(cnt_reg, gcnt_sb[0:1, pi:pi + 1])
                        cnt = cnt_reg
                    else:
                        cnt = pc * P
                    nc.gpsimd.dma_gather(
                        msg[:, :pc, :], src_ap,
                        idx_sb[:, cg * 8:(cg + pc) * 8],
                        pc * P, cnt, F,
                        queue_num=oi % 4, single_packet=False)
                    oh = ohp.tile([P, m.maxpc, P], oh_dt, tag="oh", name="oh_t")
                    nc.scalar.dma_start(
                        oh[:, :pc, :],
                        p_oh[:, cg * P:(cg + pc) * P].rearrange(
                            "p (c q) -> p c q", q=P))
                    for c in range(pc):
                        nc.tensor.matmul(
                            acc[0:F, :], msg[:, c, :], oh[:, c, :],
                            start=(done == 0),
                            stop=(done == cap_tot - 1))
                        done += 1
                    state[b][1] = done
                    if done == cap_tot:
                        epilogue(layer, b, acc)
                        del state[b]
                        if ag is not None:
                            sr = 0 if b < m.B0 else 1
                            s_done[sr] += 1
                            if s_done[sr] == (m.B0 if sr == 0 else B - m.B0):
                                ag_part(ag[0], ag[1], sr)
                assert not state

            for b in range(B):
                layer1_block(b)
                if b == m.B0 - 1:
                    ag_part(h1_loc, h1_full, 0)
                elif b == B - 1:
                    ag_part(h1_loc, h1_full, 1)
            layer_pass(2, order2, ag=(h2_loc, h2_full))
            layer_pass(3, order3)

            # per-graph window maxes were emitted inside layer 3; finish with
            # mask -> store -> AllGather -> cross-core max.
            for fh in range(2):
                nc.vector.tensor_tensor(out=pooled[fh][:], in0=pooled[fh][:],
                                        in1=gmask_sb[:],
                                        op=mybir.AluOpType.mult)
                nc.sync.dma_start(pool_loc[fh, :, :], pooled[fh][:])
            nc.gpsimd.collective_compute(
                "AllGather", mybir.AluOpType.bypass, replica_groups=rg,
                ins=[pool_loc.ap().opt()], outs=[pool_full.ap().opt()])
            pw = const.tile([P, 2 * NCORES, G], fp32)
            nc.sync.dma_start(
                pw[:],
                pool_full.ap().rearrange("k i p g -> p (k i) g"))
            for fh in range(2):
                dstc = pool_sb[:, fh * G:(fh + 1) * G]
                nc.vector.tensor_copy(out=dstc, in_=pw[:, fh, :])
                for k in range(1, NCORES):
                    nc.vector.tensor_tensor(out=dstc, in0=dstc,
                                            in1=pw[:, 2 * k + fh, :],
                                            op=mybir.AluOpType.max)

            lg = finp.tile([P, C], fp32, tag="fin", name="lg_t")
            nc.tensor.matmul(lg[:G, :], pool_sb[:, 0:G], wl_sb[0][:],
                             start=True, stop=False)
            nc.tensor.matmul(lg[:G, :], pool_sb[:, G:2 * G], wl_sb[1][:],
                             start=False, stop=False)
            nc.tensor.matmul(lg[:G, :], ones_g[:], bl_sb[:],
                             start=False, stop=True)

            z, zs = soft[:, 0:C], soft[:, C:2 * C]
            e, ot = soft[:, 2 * C:3 * C], soft[:, 3 * C:4 * C]
            pr = soft[:, 4 * C:5 * C]
            mx, sm = soft[:, 5 * C:5 * C + 1], soft[:, 5 * C + 1:5 * C + 2]
            ls, ri = soft[:, 5 * C + 2:5 * C + 3], soft[:, 5 * C + 3:5 * C + 4]
            nc.vector.tensor_copy(out=z, in_=lg[:G, :])
            nc.vector.reduce_max(out=mx, in_=z, axis=mybir.AxisListType.X)
            nc.vector.tensor_scalar(out=zs, in0=z, scalar1=mx, scalar2=None,
                                    op0=mybir.AluOpType.subtract)
            nc.scalar.activation(e, zs, AF.Exp)
            nc.vector.reduce_sum(out=sm, in_=e, axis=mybir.AxisListType.X)
            nc.scalar.activation(ls, sm, AF.Ln)
            nc.vector.reciprocal(ri, sm)
            nc.vector.tensor_scalar(out=ot, in0=zs, scalar1=ls, scalar2=None,
                                    op0=mybir.AluOpType.subtract)
            nc.vector.tensor_scalar(out=pr, in0=e, scalar1=ri, scalar2=None,
                                    op0=mybir.AluOpType.mult)
            nc.sync.dma_start(o_out[:], ot)
            nc.sync.dma_start(o_pred[:], pr)

    nc.compile()
    return nc


def make_in_maps(m):
    w = m.weights
    shared = {"w1eff": w['w1eff'], "w2eff": w['w2eff'], "w3a": w['w3a'],
              "w3b": w['w3b'], "wl": w['wl'], "bl": w['bl']}
    return [{**shared, "idxb": c['idx'], "ohb": c['oh'], "gcnt": c['gcnt'],
             "agg0T": c['agg0T'], "gmask": c['gmask']} for c in m.cores]


def run(inputs, G=32, trace=False):
    from concourse.bass_utils import run_bass_kernel_spmd
    m = preprocess(inputs, G=G)
    nc = build(m)
    maps = make_in_maps(m)
    res = run_bass_kernel_spmd(nc, maps, list(range(NCORES)), trace=trace)
    out = np.asarray(res.results[0]["o_out"])
    pred = np.asarray(res.results[0]["o_pred"])
    return (out, pred), res


def kernel(**inputs):
    """Full-inputs -> full-output GCN forward on 8 trn2 NeuronCores."""
    from concourse.bass_utils import run_bass_kernel_spmd
    m = preprocess(inputs, G=32)
    nc = build(m)
    maps = make_in_maps(m)
    res = run_bass_kernel_spmd(nc, maps, list(range(NCORES)), trace=False)
    out = np.asarray(res.results[0]["o_out"], dtype=np.float32)
    pred = np.asarray(res.results[0]["o_pred"], dtype=np.float32)
    return (out, pred)
